# revision 5
# baseline (speedup 1.0000x reference)
"""Trainium2 Bass kernel for nn_CSRA (SS2D/Mamba-style cross-scan module).

Self-contained: builds an SPMD Bass/Tile program for 8 NeuronCores, shards
inputs host-side, runs via run_bass_kernel_spmd, reassembles the output.

Sharding: core c -> (b = c//4, dh = (c%4)//2, nh = c%2).
Every core: full pre-stage for its batch b (BN+pre_proj+in_proj+depthwise
conv, all-DI); then 4 scan-direction sections over its (d-half, n-half)
lanes (8 lanes-tiles of [128, L] each, processed in 1024-column chunks with
fp32 carries); per-chunk n-reduction via identity-matmul PSUM accumulation
on the PE; 4-direction merge via zero-masked ReduceScatter over the 4 cores
of each batch; post-stage (LN, z-gate, out_proj, reverse-mask, post_proj,
gated residual) on its L-quarter q = c%4.
"""

import os
import numpy as np
import ml_dtypes

import concourse.bass as bass
import concourse.mybir as mybir
import concourse.tile as tile
from concourse import bacc
from concourse.bass_utils import run_bass_kernel_spmd
from concourse.bass_interp import get_hw_module

F32 = mybir.dt.float32
BF16 = mybir.dt.bfloat16
AF = mybir.ActivationFunctionType
OP = mybir.AluOpType

B, C, H, W = 2, 128, 64, 64
L = H * W                      # 4096
DI, N, R, K = 256, 16, 8, 4
TH = L // 2
CH = 1024                      # scan pipeline chunk
NQ = L // 4                    # l-quarter for post stage
EPS = 1e-5

bf = lambda x: np.ascontiguousarray(x).astype(ml_dtypes.bfloat16)
f32 = lambda x: np.ascontiguousarray(x, dtype=np.float32)


def _ap(t, off, dims):
    base = t[:]
    return bass.AP(tensor=base.tensor, offset=base.offset + off,
                   ap=[base.ap[0]] + [list(d) for d in dims])


# scan-order -> source AP over a canonical [*, L] tile, chunk of `cnt` cols
# starting at scan-col j0 (H-row aligned for k1/k3).
def _xs_src(u_t, k, j0, cnt):
    if k == 0:
        return _ap(u_t, j0, [[1, cnt]])
    if k == 2:
        return _ap(u_t, L - 1 - j0, [[-1, cnt]])
    nw = cnt // H
    w0 = j0 // H
    if k == 1:   # xs1[w*64+h] = u[h*64+w]
        return _ap(u_t, w0, [[1, nw], [W, H]])
    # k == 3: xs3[w*64+h] = u[4095 - 64h - w]
    return _ap(u_t, L - 1 - w0, [[-1, nw], [-W, H]])


def build_program():
    nc = bacc.Bacc("TRN2", target_bir_lowering=False, debug=False,
                   enable_asserts=False, num_devices=8)

    def inp(name, shape, dt=F32):
        return nc.dram_tensor(name, shape, dt, kind="ExternalInput").ap()

    feature = inp("feature", [C, L], BF16)
    featq = inp("featq", [C, NQ])
    mrow = inp("mrow", [1, NQ])
    wpre = inp("wpre", [C, C], BF16)
    bpre = inp("bpre", [C, 1])
    ipw = inp("ipw", [C, 2 * DI], BF16)
    convd = inp("convd", [128, 18 * 128], BF16)
    convb = inp("convb", [128, 2])
    xw = inp("xw", [128, 2 * K * 24], BF16)
    dtw = inp("dtw", [R, K * 128], BF16)
    dtb = inp("dtb", [128, K])
    Ak = inp("Ak", [128, K * 8])
    dshalf = inp("dshalf", [128, 1])
    m01 = inp("m01", [128, 2])
    lng = inp("lng", [128, 2])
    lnb = inp("lnb", [128, 2])
    opw = inp("opw", [128, 2 * 128], BF16)
    wpost = inp("wpost", [C, C], BF16)
    bpost = inp("bpost", [C, 1])
    mscbi = inp("mscbi", [128, 2])
    gatev = inp("gatev", [128, 1])
    ident = inp("ident", [128, 128], BF16)

    out_d = nc.dram_tensor("out", [C, NQ], F32, kind="ExternalOutput").ap()

    with tile.TileContext(nc) as tc:
        with tc.tile_pool(name="cn", bufs=1) as cn, \
             tc.tile_pool(name="wk", bufs=1) as wk, \
             tc.tile_pool(name="sc8", bufs=1) as sc8, \
             tc.tile_pool(name="d2", bufs=2) as d2, \
             tc.tile_pool(name="ps", bufs=2, space="PSUM") as ps, \
             tc.tile_pool(name="dram", bufs=1, space="DRAM") as dram:

            def cload(ap_in, shape, dt=F32, tag=None):
                t = cn.tile(shape, dt, tag=tag or ap_in.tensor.name,
                            name=tag or ap_in.tensor.name)
                nc.sync.dma_start(t[:], ap_in)
                return t

            wpre_t = cload(wpre, [C, C], BF16)
            bpre_t = cload(bpre, [C, 1])
            ipw_t = cload(ipw, [C, 2 * DI], BF16)
            convd_t = cload(convd, [128, 18 * 128], BF16)
            convb_t = cload(convb, [128, 2])
            xw_t = cload(xw, [128, 2 * K * 24], BF16)
            dtw_t = cload(dtw, [R, K * 128], BF16)
            dtb_t = cload(dtb, [128, K])
            Ak_t = cload(Ak, [128, K * 8])
            dsh_t = cload(dshalf, [128, 1])
            m01_t = cload(m01, [128, 2])
            lng_t = cload(lng, [128, 2])
            lnb_t = cload(lnb, [128, 2])
            opw_t = cload(opw, [128, 2 * 128], BF16)
            wpost_t = cload(wpost, [C, C], BF16)
            bpost_t = cload(bpost, [C, 1])
            mscbi_t = cload(mscbi, [128, 2])
            gate_t = cload(gatev, [128, 1])
            id_t = cload(ident, [128, 128], BF16)
            ones_t = cn.tile([128, 1], BF16, tag="ones", name="ones")
            nc.vector.memset(ones_t[:], 1.0)
            onesr_t = cn.tile([1, 128], BF16, tag="onesr", name="onesr")
            nc.vector.memset(onesr_t[:], 1.0)
            eps_t = cn.tile([128, 1], F32, tag="epsc", name="epsc")
            nc.vector.memset(eps_t[:], EPS)

            # =========== PRE-STAGE (full DI, this core's batch) ===========
            feat16 = d2.tile([C, L], BF16, tag="xdbl", name="xdbl")
            nc.sync.dma_start(feat16[:, 0:TH], feature[:, 0:TH])
            nc.sync.dma_start(feat16[:, TH:L], feature[:, TH:L])

            feat2 = sc8.tile([C, L], BF16, tag="sc8", name="sc8")
            for c2 in range(0, L, 1024):
                pb = ps.tile([128, 1024], F32, tag="big2", name="big2")
                for c5 in range(0, 1024, 512):
                    nc.tensor.matmul(pb[:, c5:c5 + 512], wpre_t[:],
                                     feat16[:, c2 + c5:c2 + c5 + 512],
                                     start=True, stop=True)
                nc.scalar.activation(feat2[:, c2:c2 + 1024], pb[:], AF.Gelu,
                                     bias=bpre_t[:, 0:1], scale=1.0)

            # x = in_proj(feat2)[:DI] written into zero-padded conv inputs
            HP, WP2 = H + 2, W + 2
            pads = []
            for dh in range(2):
                pad = d2.tile([128, HP * WP2], BF16, tag="rs", name="rs")
                nc.gpsimd.memset(pad[:], 0.0)
                pads.append(pad)
            for dh in range(2):
                for c2 in range(0, L, 1024):
                    pb = ps.tile([128, 1024], F32, tag="big2", name="big2")
                    for c5 in range(0, 1024, 512):
                        nc.tensor.matmul(pb[:, c5:c5 + 512],
                                         ipw_t[:, dh * 128:(dh + 1) * 128],
                                         feat2[:, c2 + c5:c2 + c5 + 512],
                                         start=True, stop=True)
                    h0 = c2 // W
                    nc.scalar.copy(
                        _ap(pads[dh], (h0 + 1) * WP2 + 1, [[WP2, 16], [1, W]]),
                        pb[:])

            # depthwise conv + bias + silu -> u16[dh]
            u16 = [wk.tile([128, L], BF16, tag=f"u{dh}", name=f"u{dh}")
                   for dh in range(2)]
            for dh in range(2):
                for blk in range(0, L, 1024):
                    pb = ps.tile([128, 1024], F32, tag="big2", name="big2")
                    for tap in range(9):
                        dy, dx = tap // 3, tap % 3
                        for sub in range(0, 1024, 512):
                            h0 = (blk + sub) // W
                            src = _ap(pads[dh], (h0 + dy) * WP2 + dx,
                                      [[WP2, 8], [1, W]])
                            nc.tensor.matmul(
                                pb[:, sub:sub + 512],
                                convd_t[:, (dh * 9 + tap) * 128:
                                        (dh * 9 + tap + 1) * 128],
                                src, start=(tap == 0), stop=(tap == 8))
                    nc.scalar.activation(u16[dh][:, blk:blk + 1024], pb[:],
                                         AF.Silu, bias=convb_t[:, dh:dh + 1],
                                         scale=1.0)

            # this core's d-half lanes: uown = m0*u0 + m1*u1
            uown = wk.tile([128, L], BF16, tag="uown", name="uown")
            tmpu = d2.tile([128, L], BF16, tag="rs", name="rs")
            nc.vector.tensor_scalar(uown[:], u16[0][:], m01_t[:, 0:1], None,
                                    OP.mult)
            nc.vector.tensor_scalar(tmpu[:], u16[1][:], m01_t[:, 1:2], None,
                                    OP.mult)
            nc.vector.tensor_tensor(uown[:], uown[:], tmpu[:], OP.add)

            # y accumulator (canonical order), init = 0.5*sum_k Ds_k (.) uown
            y32 = wk.tile([128, L], F32, tag="y32", name="y32")
            nc.vector.tensor_scalar(y32[:], uown[:], dsh_t[:, 0:1], None,
                                    OP.mult)

            # =========== SCAN SECTIONS (k = 0..3) ===========
            bc_d = [dram.tile([16, L], BF16, tag=f"bc{k}", name=f"bc{k}")
                    for k in range(K)]
            for k in range(K):
                # x_dbl_k = xw_k @ xs_k -> [24, L] (rows: 8 dts, 8 B, 8 C)
                xdbl = d2.tile([24, L], BF16, tag="xdbl", name="xdbl")
                for blk in range(0, L, 1024):
                    pb = ps.tile([24, 1024], F32, tag="big2", name="big2")
                    for ci in range(0, 1024, 512):
                        for dh in range(2):
                            nc.tensor.matmul(
                                pb[:, ci:ci + 512],
                                xw_t[:, (dh * K + k) * 24:(dh * K + k + 1) * 24],
                                _xs_src(u16[dh], k, blk + ci, 512),
                                start=(dh == 0), stop=(dh == 1))
                    nc.scalar.copy(xdbl[:, blk:blk + 1024], pb[:])
                nc.sync.dma_start(bc_d[k][:], xdbl[8:24, :])

                # delta_k = softplus(dt_proj_k(dts) + dtb_k)  [128, L] f32
                dl32 = d2.tile([128, L], F32, tag="dl32", name="dl32", bufs=1)
                for c2 in range(0, L, 1024):
                    pb = ps.tile([128, 1024], F32, tag="big2", name="big2")
                    for c5 in range(0, 1024, 512):
                        nc.tensor.matmul(pb[:, c5:c5 + 512],
                                         dtw_t[:, k * 128:(k + 1) * 128],
                                         xdbl[0:8, c2 + c5:c2 + c5 + 512],
                                         start=True, stop=True)
                    nc.scalar.activation(dl32[:, c2:c2 + 1024], pb[:], AF.Exp,
                                         bias=dtb_t[:, k:k + 1], scale=1.0)
                for c2 in range(0, L, 2048):
                    nc.scalar.activation(dl32[:, c2:c2 + 2048],
                                         dl32[:, c2:c2 + 2048], AF.Ln,
                                         bias=1.0, scale=1.0)

                # dtu_k = delta_k * xs_k(own lanes), per t-half
                dtu = []
                for t in range(2):
                    dt_h = d2.tile([128, TH], BF16, tag="dtu", name="dtu")
                    nc.vector.tensor_tensor(dt_h[:], dl32[:, t * TH:(t + 1) * TH],
                                            _xs_src(uown, k, t * TH, TH), OP.mult)
                    dtu.append(dt_h)

                carries = [None] * 8
                for c in range(L // CH):
                    red = ps.tile([128, CH], F32, tag="red", name="red")
                    for n in range(8):
                        brep = d2.tile([128, CH], BF16, tag="brep", name="brep",
                                       bufs=3)
                        nc.sync.dma_start(
                            brep[:],
                            bass.AP(tensor=bc_d[k][:].tensor,
                                    offset=bc_d[k][:].offset + n * L + c * CH,
                                    ap=[[0, 128], [1, CH]]))
                        crep = d2.tile([128, CH], BF16, tag="crep", name="crep",
                                       bufs=3)
                        nc.sync.dma_start(
                            crep[:],
                            bass.AP(tensor=bc_d[k][:].tensor,
                                    offset=bc_d[k][:].offset + (8 + n) * L + c * CH,
                                    ap=[[0, 128], [1, CH]]))
                        a16 = d2.tile([128, CH], BF16, tag="a16", name="a16",
                                      bufs=3)
                        nc.scalar.activation(a16[:], dl32[:, c * CH:(c + 1) * CH],
                                             AF.Exp, bias=0.0,
                                             scale=Ak_t[:, k * 8 + n:k * 8 + n + 1])
                        b16 = d2.tile([128, CH], BF16, tag="b16", name="b16",
                                      bufs=3)
                        nc.vector.tensor_tensor(
                            b16[:], dtu[c // 2][:, (c % 2) * CH:(c % 2 + 1) * CH],
                            brep[:], OP.mult)
                        h16 = d2.tile([128, CH], BF16, tag="h16", name="h16",
                                      bufs=3)
                        init = 0.0 if c == 0 else carries[n][:, 0:1]
                        nc.vector.tensor_tensor_scan(h16[:], a16[:], b16[:],
                                                     init, OP.mult, OP.add)
                        if c < L // CH - 1:
                            cr = d2.tile([128, 1], F32, tag="carry",
                                         name="carry", bufs=16)
                            nc.vector.tensor_copy(cr[:], h16[:, CH - 1:CH])
                            carries[n] = cr
                        p16 = d2.tile([128, CH], BF16, tag="p16", name="p16",
                                      bufs=3)
                        nc.vector.tensor_tensor(p16[:], h16[:], crep[:], OP.mult)
                        for c5 in range(0, CH, 512):
                            nc.tensor.matmul(red[:, c5:c5 + 512], id_t[:],
                                             p16[:, c5:c5 + 512],
                                             start=(n == 0), stop=(n == 7))
                    dst = _xs_src(y32, k, c * CH, CH)
                    nc.vector.tensor_tensor(dst, red[:], dst, OP.add)

            # ---- post-phase precompute (fills the collective wait) ----
            featq32 = d2.tile([128, NQ], F32, tag="fq32", name="fq32", bufs=1)
            nc.sync.dma_start(featq32[:], featq)
            featq16 = d2.tile([128, NQ], BF16, tag="pe", name="pe", bufs=2)
            nc.vector.tensor_copy(featq16[:], featq32[:])
            fq2 = d2.tile([128, NQ], BF16, tag="pe", name="pe", bufs=2)
            pb = ps.tile([128, 1024], F32, tag="big2", name="big2")
            for c5 in range(0, NQ, 512):
                nc.tensor.matmul(pb[:, c5:c5 + 512], wpre_t[:],
                                 featq16[:, c5:c5 + 512], start=True, stop=True)
            nc.scalar.activation(fq2[:], pb[:], AF.Gelu,
                                 bias=bpre_t[:, 0:1], scale=1.0)
            zq = []
            for dh in range(2):
                pb = ps.tile([128, 1024], F32, tag="big2", name="big2")
                for c5 in range(0, NQ, 512):
                    nc.tensor.matmul(pb[:, c5:c5 + 512],
                                     ipw_t[:, (2 + dh) * 128:(3 + dh) * 128],
                                     fq2[:, c5:c5 + 512], start=True, stop=True)
                z = d2.tile([128, NQ], BF16, tag="zq", name="zq")
                nc.scalar.activation(z[:], pb[:], AF.Silu)
                zq.append(z)
            mq = d2.tile([128, NQ], F32, tag="brep", name="brep", bufs=3)
            nc.sync.dma_start(mq[:], bass.AP(
                tensor=mrow.tensor, offset=mrow.offset, ap=[[0, 128], [1, NQ]]))
            m16 = d2.tile([128, NQ], BF16, tag="pe", name="pe", bufs=2)
            nc.scalar.activation(m16[:], mq[:], AF.Sigmoid,
                                 bias=mscbi_t[:, 1:2], scale=mscbi_t[:, 0:1])

            # =========== MERGE: masked cast + ReduceScatter ===========
            rs_in = dram.tile([8, 128, NQ], BF16, tag="rsin", name="rsin")
            rs_out = dram.tile([2, 128, NQ], BF16, tag="rsout", name="rsout")
            for j in range(2):
                ym = d2.tile([128, L], BF16, tag="rs", name="rs")
                nc.vector.tensor_scalar(ym[:], y32[:], m01_t[:, j:j + 1], None,
                                        OP.mult)
                for q in range(4):
                    nc.sync.dma_start(rs_in[2 * q + j],
                                      ym[:, q * NQ:(q + 1) * NQ])
            nc.gpsimd.collective_compute(
                "ReduceScatter", OP.add,
                replica_groups=[[0, 1, 2, 3], [4, 5, 6, 7]],
                ins=[rs_in.opt()], outs=[rs_out.opt()])

            ysum = []
            for j in range(2):
                t = d2.tile([128, NQ], BF16, tag="a16", name="a16", bufs=3)
                nc.sync.dma_start(t[:], rs_out[j])
                ysum.append(t)

            # =========== POST-STAGE (this core's l-quarter) ===========
            sq = []
            for j in range(2):
                s = d2.tile([128, NQ], BF16, tag="h16", name="h16", bufs=3)
                nc.scalar.activation(s[:], ysum[j][:], AF.Square)
                sq.append(s)
            mu = d2.tile([1, NQ], F32, tag="brep", name="brep", bufs=3)
            e2 = d2.tile([1, NQ], F32, tag="crep", name="crep", bufs=3)
            for which, tiles in ((0, ysum), (1, sq)):
                for c5 in range(0, NQ, 512):
                    pc = ps.tile([1, 512], F32, tag="red", name="red")
                    for j in range(2):
                        nc.tensor.matmul(pc[:], ones_t[:],
                                         tiles[j][:, c5:c5 + 512],
                                         start=(j == 0), stop=(j == 1))
                    dst = mu if which == 0 else e2
                    nc.scalar.activation(dst[:, c5:c5 + 512], pc[:], AF.Copy,
                                         bias=0.0, scale=1.0 / 256.0)
            mu2 = d2.tile([1, NQ], F32, tag="b16", name="b16", bufs=3)
            nc.scalar.activation(mu2[:], mu[:], AF.Square)
            var = d2.tile([1, NQ], F32, tag="dtu", name="dtu")
            nc.vector.tensor_tensor(var[:], e2[:], mu2[:], OP.subtract)
            sd = d2.tile([1, NQ], F32, tag="brep", name="brep", bufs=3)
            nc.scalar.activation(sd[:], var[:], AF.Ln, bias=eps_t[0:1, 0:1],
                                 scale=1.0)
            inv = d2.tile([1, NQ], F32, tag="h16", name="h16", bufs=3)
            nc.scalar.activation(inv[:], sd[:], AF.Exp, bias=0.0, scale=-0.5)
            qrow = d2.tile([1, NQ], F32, tag="crep", name="crep", bufs=3)
            nc.vector.tensor_tensor(qrow[:], mu[:], inv[:], OP.mult)

            inv16 = d2.tile([1, NQ], BF16, tag="srow", name="srow")
            nc.scalar.copy(inv16[:], inv[:])
            q16 = d2.tile([1, NQ], BF16, tag="srow", name="srow")
            nc.scalar.copy(q16[:], qrow[:])
            invrep = ps.tile([128, NQ], F32, tag="big2", name="big2")
            qrep = ps.tile([128, NQ], F32, tag="big2", name="big2")
            for c5 in range(0, NQ, 512):
                nc.tensor.matmul(invrep[:, c5:c5 + 512], onesr_t[:],
                                 inv16[0:1, c5:c5 + 512], start=True, stop=True)
                nc.tensor.matmul(qrep[:, c5:c5 + 512], onesr_t[:],
                                 q16[0:1, c5:c5 + 512], start=True, stop=True)

            ym16 = []
            for j in range(2):
                t1 = d2.tile([128, NQ], F32, tag="dl32", name="dl32", bufs=1)
                nc.vector.tensor_tensor(t1[:], ysum[j][:], invrep[:], OP.mult)
                nc.vector.tensor_tensor(t1[:], t1[:], qrep[:], OP.subtract)
                yl = d2.tile([128, NQ], BF16, tag="brep", name="brep", bufs=3)
                nc.vector.tensor_scalar(yl[:], t1[:], lng_t[:, j:j + 1],
                                        lnb_t[:, j:j + 1], OP.mult, OP.add)
                ym = d2.tile([128, NQ], BF16, tag="h16", name="h16", bufs=3)
                nc.vector.tensor_tensor(ym[:], yl[:], zq[j][:], OP.mult)
                ym16.append(ym)

            att = d2.tile([128, NQ], BF16, tag="xdbl", name="xdbl")
            for c5 in range(0, NQ, 512):
                pc = ps.tile([128, 512], F32, tag="red", name="red")
                for j in range(2):
                    nc.tensor.matmul(pc[:], opw_t[:, j * 128:(j + 1) * 128],
                                     ym16[j][:, c5:c5 + 512],
                                     start=(j == 0), stop=(j == 1))
                nc.vector.tensor_tensor(att[:, c5:c5 + 512], pc[:],
                                        m16[:, c5:c5 + 512], OP.mult)

            ref32 = d2.tile([128, NQ], F32, tag="rs", name="rs")
            pb = ps.tile([128, 1024], F32, tag="big2", name="big2")
            for c5 in range(0, NQ, 512):
                nc.tensor.matmul(pb[:, c5:c5 + 512], wpost_t[:],
                                 att[:, c5:c5 + 512], start=True, stop=True)
            nc.scalar.activation(ref32[:], pb[:], AF.Gelu,
                                 bias=bpost_t[:, 0:1], scale=1.0)

            o32 = d2.tile([128, NQ], F32, tag="dl32", name="dl32", bufs=1)
            nc.vector.scalar_tensor_tensor(o32[:], ref32[:], gate_t[:, 0:1],
                                           featq32[:], OP.mult, OP.add)
            nc.sync.dma_start(out_d, o32[:])

    nc.compile()
    nc.m = get_hw_module(nc.m)
    return nc


def make_in_maps(inputs):
    fe = f32(inputs["feature"])
    mask = f32(inputs["mask_pred"])
    s1 = inputs["bn1_gamma"] / np.sqrt(inputs["bn1_var"] + EPS)
    t1 = inputs["bn1_beta"] - inputs["bn1_mean"] * s1
    W1 = inputs["pre_w"] * s1[None, :]
    b1 = inputs["pre_w"] @ t1
    s2 = inputs["pre_g"] / np.sqrt(inputs["pre_v"] + EPS)
    t2 = inputs["pre_b"] - inputs["pre_m"] * s2
    Wpre = W1 * s2[:, None]
    bpre_v = b1 * s2 + t2
    sp = inputs["post_g"] / np.sqrt(inputs["post_v"] + EPS)
    tp = inputs["post_b"] - inputs["post_m"] * sp
    Wpost = inputs["post_w"] * sp[:, None]
    sm = inputs["mbn_g"][0] / np.sqrt(inputs["mbn_v"][0] + EPS)
    tm = inputs["mbn_b"][0] - inputs["mbn_m"][0] * sm
    A = -np.exp(f32(inputs["A_logs"])).reshape(K, DI, N)
    Ds3 = f32(inputs["Ds"]).reshape(K, DI)
    xw_full = f32(inputs["x_proj_w"])
    dtw_full = f32(inputs["dt_proj_w"])
    dtb_full = f32(inputs["dt_proj_b"])
    ipw_full = f32(inputs["in_proj_w"])
    conv_w = f32(inputs["conv_w"])
    opw_full = f32(inputs["out_proj_w"])

    convd = np.zeros((128, 18 * 128), np.float32)
    for dh in range(2):
        for tap in range(9):
            blk = convd[:, (dh * 9 + tap) * 128:(dh * 9 + tap + 1) * 128]
            np.fill_diagonal(blk, conv_w[dh * 128:(dh + 1) * 128,
                                         tap // 3, tap % 3])

    opw = np.zeros((128, 256), np.float32)
    for j in range(2):
        opw[:, j * 128:(j + 1) * 128] = opw_full[:, j * 128:(j + 1) * 128].T
    lng = np.stack([inputs["out_ln_g"][:128], inputs["out_ln_g"][128:]], 1)
    lnb = np.stack([inputs["out_ln_b"][:128], inputs["out_ln_b"][128:]], 1)

    common = dict(
        wpre=bf(Wpre.T), bpre=f32(bpre_v)[:, None],
        ipw=bf(ipw_full.T), convd=bf(convd),
        convb=f32(np.stack([inputs["conv_b"][:128], inputs["conv_b"][128:]], 1)),
        lng=f32(lng), lnb=f32(lnb), opw=bf(opw),
        wpost=bf(Wpost.T), bpost=f32(tp)[:, None],
        mscbi=f32(np.tile(np.array([[-sm, -tm]], np.float32), (128, 1))),
        gatev=f32(np.full((128, 1), inputs["gate"][0], np.float32)),
        ident=bf(np.eye(128, dtype=np.float32)),
    )

    in_maps = []
    for c in range(8):
        b, dh, nh, q = c // 4, (c % 4) // 2, c % 2, c % 4
        dsl = slice(dh * 128, (dh + 1) * 128)
        sel = np.r_[0:R, R + nh * 8:R + nh * 8 + 8,
                    R + N + nh * 8:R + N + nh * 8 + 8]
        xw_c = np.zeros((128, 2 * K * 24), np.float32)
        for dh2 in range(2):
            for k in range(K):
                xw_c[:, (dh2 * K + k) * 24:(dh2 * K + k + 1) * 24] = \
                    xw_full[k][sel][:, dh2 * 128:(dh2 + 1) * 128].T
        dtw_c = np.zeros((R, K * 128), np.float32)
        for k in range(K):
            dtw_c[:, k * 128:(k + 1) * 128] = dtw_full[k, dsl, :].T
        m01c = np.zeros((128, 2), np.float32)
        m01c[:, dh] = 1.0
        fb = fe[b].reshape(C, L)
        m = dict(common)
        m.update(
            feature=bf(fb),
            featq=f32(fb[:, q * NQ:(q + 1) * NQ]),
            mrow=f32(mask[b, 0].reshape(1, L)[:, q * NQ:(q + 1) * NQ]),
            xw=bf(xw_c), dtw=bf(dtw_c),
            dtb=f32(dtb_full[:, dsl].T),
            Ak=f32(A[:, dsl, nh * 8:nh * 8 + 8].transpose(1, 0, 2)
                   .reshape(128, K * 8)),
            dshalf=f32(0.5 * Ds3[:, dsl].sum(0))[:, None],
            m01=m01c,
        )
        in_maps.append(m)
    return in_maps


_CACHE = {}


def kernel(**inputs):
    if "nc" not in _CACHE:
        _CACHE["nc"] = build_program()
    nc = _CACHE["nc"]
    in_maps = make_in_maps(inputs)
    res = run_bass_kernel_spmd(nc, in_maps, list(range(8)))
    out = np.empty((B, C, H, W), np.float32)
    for c in range(8):
        b, q = c // 4, c % 4
        out[b].reshape(C, L)[:, q * NQ:(q + 1) * NQ] = res.results[c]["out"]
    return out
